# revision 12
# baseline (speedup 1.0000x reference)
"""Trainium2 Bass kernel: batched channel-attention (Gram-matrix form).

Self-contained: builds the Bass/Tile program, shards the full inputs over
8 NeuronCores (one batch element each), and gathers the full output.

Design notes:
- Host passes x twice in fp16: natural layout [C, N] (phase-B moving
  operand) and a transposed, 16-sub-packed layout with ones columns
  appended (phase-A Gram operands), padded so every sub-row is 32B
  aligned.  Same total input bytes as the original fp32 x, but no PE
  transposes are needed on device.
- att = W1 G W2^T + rank-1 terms, G = [x|1][x|1]^T accumulated over
  128-column n-subs.  The big C x C products run in fp32 (fp16/f32r
  there visibly hurts accuracy); the tiny s-projections and rank-1
  terms run in fp16.
- softmax: unnormalized exp, scaled by 1/rowsum on the Scalar engine
  (activation Copy with a per-partition scale) before the PE transpose;
  the residual +x becomes +I on the diagonal attT blocks, so phase B is
  a pure matmul + PSUM->SBUF copy + DMA out.
- Output is written fp16 (half the store traffic); the host upcasts.
"""

from contextlib import ExitStack

import concourse.bass as bass
import concourse.tile as tile
from concourse import bacc, mybir
from concourse.masks import make_identity

F32 = mybir.dt.float32
F16 = mybir.dt.float16

C = 256
CH = 128  # half of C, = partition count
N = 16384
NSUB = N // 128          # 128 n-subs
GRP = 8                  # xt DMA groups
KSUB = NSUB // GRP       # 16 subs per group
XTW = C + 2              # 258: channels + two ones columns
WPAD = 272               # padded sub-row (272*2B = 544B = 17*32B, 32B-aligned)
EXPF = mybir.ActivationFunctionType.Exp
COPYF = mybir.ActivationFunctionType.Copy
ALU = mybir.AluOpType


def build_nc(
    out_chunks=(512, 512, 1024, 2048, 2048, 2048, 2048, 2048, 2048, 1024, 512, 512),
    nt=512,
    av_bufs=4,
    out_bufs=3,
    xt_bufs=8,
):
    assert sum(out_chunks) == N
    nc = bacc.Bacc(None, target_bir_lowering=False)

    x16 = nc.dram_tensor("x16", [C, N], F16, kind="ExternalInput")
    # packed transposed x: row p, col s*WPAD+c holds x[c, 128s+p],
    # cols 256:258 pre-filled with ones (row-sum trick), 258:272 zero pad
    xt4 = nc.dram_tensor("xt4", [128, NSUB * WPAD], F16, kind="ExternalInput")
    w1t = nc.dram_tensor("w1t", [C, C], F32, kind="ExternalInput")
    w2t = nc.dram_tensor("w2t", [C, C], F32, kind="ExternalInput")
    b1 = nc.dram_tensor("b1", [1, C], F32, kind="ExternalInput")
    b2 = nc.dram_tensor("b2", [1, C], F32, kind="ExternalInput")
    y = nc.dram_tensor("y", [C, N], F16, kind="ExternalOutput")

    ostarts = []
    pos = 0
    for w in out_chunks:
        ostarts.append(pos)
        pos += w

    with tile.TileContext(nc) as tc, ExitStack() as ctx:
        consts = ctx.enter_context(tc.tile_pool(name="consts", bufs=1))
        xfp = ctx.enter_context(tc.tile_pool(name="xf", bufs=1))
        small = ctx.enter_context(tc.tile_pool(name="small", bufs=1))

        # ---- Phase A: G = [x|1][x|1]^T from the packed transposed layout ----
        # All input loads share the SWDGE ring in program order: xt groups
        # first (phase-A critical path), then the natural-layout xf chunks.
        # xt_bufs=GRP keeps every group resident so the ring never stalls on
        # Gram-matmul backpressure.  Emitted first so the scheduler gives the
        # ring to the xt loads before any constant setup.
        xfc = [[None] * len(out_chunks) for _ in range(2)]
        # ramp group sizes: a small first DMA lands quickly so the Gram
        # matmuls start ~3us earlier; steady-state groups are 16 subs (1 MB)
        xt_groups = (4, 8, 16, 16, 16, 16, 16, 16, 16, 4)
        assert sum(xt_groups) == NSUB
        with tc.tile_pool(name="psum_g", bufs=1, space="PSUM") as pg:
            g_ps = [pg.tile([CH, XTW], F32, name=f"g{h}", tag=f"g{h}") for h in range(2)]
            with tc.tile_pool(name="xt", bufs=len(xt_groups)) as xt_pool:
                s0 = 0
                for gk in xt_groups:
                    xt_t = xt_pool.tile([128, 16 * WPAD], F16, name="xt", tag="xt")
                    nc.gpsimd.dma_start(
                        xt_t[:, 0:gk * WPAD],
                        xt4[:, s0 * WPAD:(s0 + gk) * WPAD],
                    )
                    for k in range(gk):
                        s = s0 + k
                        base = k * WPAD
                        for h in range(2):
                            nc.tensor.matmul(
                                g_ps[h][:],
                                xt_t[:, base + h * CH: base + (h + 1) * CH],
                                xt_t[:, base: base + XTW],
                                start=(s == 0),
                                stop=(s == NSUB - 1),
                            )
                    s0 += gk
            g_sb = [small.tile([CH, XTW], F32, name=f"gsb{h}", tag=f"gsb{h}") for h in range(2)]
            s16 = small.tile([CH, 4], F16, name="s16", tag="s16")
            nc.vector.tensor_copy(g_sb[0][:], g_ps[0][:])
            nc.scalar.copy(g_sb[1][:], g_ps[1][:])
            for h in range(2):
                nc.vector.tensor_copy(s16[:, 2 * h:2 * h + 2], g_ps[h][:, C:C + 2])

        # natural-layout x for phase B, queued behind the xt loads on the
        # same ring so it never steals bandwidth from the critical path
        for j, w in enumerate(out_chunks):
            sl = slice(ostarts[j], ostarts[j] + w)
            for h in range(2):
                t = xfp.tile([CH, w], F16, name=f"xf{h}_{j}", tag=f"xf{h}_{j}")
                xfc[h][j] = t
                nc.gpsimd.dma_start(t[:], x16[h * CH:(h + 1) * CH, sl])

        # constants (emitted after the loads so they don't delay the ring)
        ident = consts.tile([128, 128], F16, name="ident", tag="ident")
        make_identity(nc, ident[:])

        # preload the exp table set early (off the critical path)
        dummy = small.tile([1, 2], F32, name="dummy", tag="dummy")
        nc.vector.memset(dummy[:], 0.0)
        nc.scalar.activation(dummy[:], dummy[:], EXPF)

        # weights / biases
        w1_sb = [consts.tile([CH, C], F32, name=f"w1_{h}", tag=f"w1_{h}") for h in range(2)]
        w2_sb = [consts.tile([CH, C], F32, name=f"w2_{h}", tag=f"w2_{h}") for h in range(2)]
        w1_16 = [consts.tile([CH, C], F16, name=f"w1h{h}", tag=f"w1h{h}") for h in range(2)]
        w2_16 = [consts.tile([CH, C], F16, name=f"w2h{h}", tag=f"w2h{h}") for h in range(2)]
        for h in range(2):
            nc.scalar.dma_start(w1_sb[h][:], w1t[h * CH:(h + 1) * CH, :])
            nc.scalar.dma_start(w2_sb[h][:], w2t[h * CH:(h + 1) * CH, :])
            nc.vector.tensor_copy(w1_16[h][:], w1_sb[h][:])
            nc.vector.tensor_copy(w2_16[h][:], w2_sb[h][:])
        b1_row = small.tile([1, C], F16, name="b1r", tag="b1r")
        b2_row = small.tile([1, C], F16, name="b2r", tag="b2r")
        b1f_row = small.tile([1, C], F32, name="b1fr", tag="b1fr")
        b2f_row = small.tile([1, C], F32, name="b2fr", tag="b2fr")
        nc.scalar.dma_start(b1f_row[:], b1[:])
        nc.scalar.dma_start(b2f_row[:], b2[:])
        nc.vector.tensor_copy(b1_row[:], b1f_row[:])
        nc.vector.tensor_copy(b2_row[:], b2f_row[:])

        # ---- C x C algebra ----
        with tc.tile_pool(name="psum_alg", bufs=1, space="PSUM") as pa:
            # U^T blocks: u_ps[d] = (W1 G)^T rows for d-half (fp32)
            u_ps = [pa.tile([CH, C], F32, name=f"u{d}", tag=f"u{d}") for d in range(2)]
            for d in range(2):
                for h in range(2):
                    nc.tensor.matmul(
                        u_ps[d][:],
                        g_sb[h][:, d * CH:(d + 1) * CH],
                        w1_sb[h][:],
                        start=(h == 0), stop=(h == 1),
                    )
            u_sb = [small.tile([CH, C], F32, name=f"usb{d}", tag=f"usb{d}") for d in range(2)]
            nc.vector.tensor_copy(u_sb[0][:], u_ps[0][:])
            nc.scalar.copy(u_sb[1][:], u_ps[1][:])

            # s-vector projections for the rank-1 terms (tiny, fp16)
            w1s_ps = pa.tile([2, C], F32, name="w1s", tag="w1s")
            w2s_ps = pa.tile([2, C], F32, name="w2s", tag="w2s")
            for h in range(2):
                nc.tensor.matmul(
                    w1s_ps[:], s16[:, 2 * h:2 * h + 2], w1_16[h][:],
                    start=(h == 0), stop=(h == 1),
                )
            for h in range(2):
                nc.tensor.matmul(
                    w2s_ps[:], s16[:, 2 * h:2 * h + 2], w2_16[h][:],
                    start=(h == 0), stop=(h == 1),
                )
            w1s_row = small.tile([1, C], F16, name="w1sr", tag="w1sr")
            w2sn_row = small.tile([1, C], F16, name="w2snr", tag="w2snr")
            nc.vector.tensor_copy(w1s_row[:], w1s_ps[0:1, :])
            # (W2 s) + N * b2
            nc.vector.scalar_tensor_tensor(
                w2sn_row[:], b2f_row[:], float(N), w2s_ps[0:1, :],
                op0=ALU.mult, op1=ALU.add,
            )

            att_ps = [pa.tile([CH, C], F32, name=f"att{o}", tag=f"att{o}") for o in range(2)]
            attt_ps = [pa.tile([CH, C], F16, name=f"atp{d}", tag=f"atp{d}") for d in range(2)]
            negmax = [small.tile([CH, 1], F32, name=f"nm{o}", tag=f"nm{o}") for o in range(2)]
            rowsum = [small.tile([CH, 1], F32, name=f"rs{o}", tag=f"rs{o}") for o in range(2)]
            rowinv = [small.tile([CH, 1], F32, name=f"ri{o}", tag=f"ri{o}") for o in range(2)]
            exp_sb = [small.tile([CH, C], F16, name=f"exp{o}", tag=f"exp{o}") for o in range(2)]
            exps = [small.tile([CH, C], F16, name=f"exs{o}", tag=f"exs{o}") for o in range(2)]
            # attT blocks: attt_sb4[d][o] = attT[d-half rows, o-half cols]
            attt_sb4 = [
                [small.tile([CH, CH], F16, name=f"at{d}{o}", tag=f"at{d}{o}") for o in range(2)]
                for d in range(2)
            ]

            for o in range(2):
                osl = slice(o * CH, (o + 1) * CH)
                for d in range(2):
                    nc.tensor.matmul(
                        att_ps[o][:], u_sb[d][:, osl], w2_sb[d][:],
                        start=(d == 0), stop=False,
                    )
                # tiny rank-1 terms last (fp16)
                nc.tensor.matmul(
                    att_ps[o][:], w1s_row[:, osl], b2_row[:],
                    start=False, stop=False,
                )
                nc.tensor.matmul(
                    att_ps[o][:], b1_row[:, osl], w2sn_row[:],
                    start=False, stop=True,
                )
                # softmax: unnormalized exp, then scale rows by 1/rowsum
                nc.vector.reduce_max(
                    negmax[o][:], att_ps[o][:], axis=mybir.AxisListType.X,
                    negate=True,
                )
                nc.scalar.activation(
                    exp_sb[o][:], att_ps[o][:], EXPF,
                    bias=negmax[o][:], scale=1.0,
                    accum_out=rowsum[o][:],
                )
                nc.vector.reciprocal(rowinv[o][:], rowsum[o][:])
                nc.scalar.activation(
                    exps[o][:], exp_sb[o][:], COPYF, scale=rowinv[o][:],
                )
                # attT o-column blocks + I on the diagonal block
                for d in range(2):
                    nc.tensor.transpose(
                        attt_ps[d][:, osl],
                        exps[o][:, d * CH:(d + 1) * CH],
                        ident[:],
                    )
                    if o == d:
                        nc.vector.tensor_add(
                            attt_sb4[d][o][:], attt_ps[d][:, osl], ident[:]
                        )
                    else:
                        nc.scalar.copy(attt_sb4[d][o][:], attt_ps[d][:, osl])

        # ---- Phase B: out = (attT)^T @ xf, pure matmul + evacuate + DMA ----
        # o-outer: the o=0 output half only needs the o=0 softmax half, so
        # stores start while the o=1 algebra is still in flight.  Within a
        # chunk the d loop is outermost so the stationary operand is loaded
        # once per d instead of once per matmul.
        with tc.tile_pool(name="psum_b", bufs=av_bufs, space="PSUM") as pb, \
             tc.tile_pool(name="outp", bufs=out_bufs) as op:
            # av tiles span 2 PSUM banks (1024 fp32); matmuls fill them in
            # 512-col sub-slices (one bank per matmul), then a single wide
            # copy evacuates the whole tile (fewer per-op bubbles).
            avw = 2 * nt
            ci = 0
            for o in range(2):
                osl = slice(o * CH, (o + 1) * CH)
                for j, oc in enumerate(out_chunks):
                    ob = op.tile([CH, max(out_chunks)], F16, name=f"ob{o}", tag=f"ob{o}")
                    for a0 in range(0, oc, avw):
                        aw = min(avw, oc - a0)
                        av = pb.tile([CH, avw], F32, name="av", tag="av")
                        for d in range(2):
                            for t0 in range(0, aw, nt):
                                tw = min(nt, aw - t0)
                                nc.tensor.matmul(
                                    av[:, t0:t0 + tw],
                                    attt_sb4[d][o][:],
                                    xfc[d][j][:, a0 + t0:a0 + t0 + tw],
                                    start=(d == 0), stop=(d == 1),
                                )
                        if ci % 2 == 0:
                            nc.vector.tensor_copy(ob[:, a0:a0 + aw], av[:, 0:aw])
                        else:
                            nc.scalar.copy(ob[:, a0:a0 + aw], av[:, 0:aw])
                        ci += 1
                    eng = nc.sync if (j + o) % 2 == 0 else nc.scalar
                    eng.dma_start(y[osl, ostarts[j]:ostarts[j] + oc], ob[:, 0:oc])

    nc.compile()
    return nc


# ---------------------------------------------------------------------------
# Host-side entry point: shard batch over the 8 NeuronCores, run, gather.
# ---------------------------------------------------------------------------

import numpy as np

_NC_CACHE = {}


def _get_nc():
    if "nc" not in _NC_CACHE:
        _NC_CACHE["nc"] = build_nc()
    return _NC_CACHE["nc"]


def _prepare_in_maps(x, w1, b1, w2, b2):
    x = np.ascontiguousarray(np.asarray(x, dtype=np.float32))
    B, C_, H, W = x.shape
    n = H * W
    xb = x.reshape(B, C_, n).astype(np.float16)

    w1t = np.ascontiguousarray(np.asarray(w1, dtype=np.float32).T)
    w2t = np.ascontiguousarray(np.asarray(w2, dtype=np.float32).T)
    b1r = np.ascontiguousarray(np.asarray(b1, dtype=np.float32).reshape(1, C_))
    b2r = np.ascontiguousarray(np.asarray(b2, dtype=np.float32).reshape(1, C_))

    in_maps = []
    for i in range(B):
        xi = xb[i]                                   # [256, 16384] fp16
        # packed transposed layout: [p, s, c] = x[c, 128s + p]
        core = xi.reshape(C_, n // 128, 128).transpose(2, 1, 0)
        xt4 = np.zeros((128, n // 128, WPAD), dtype=np.float16)
        xt4[..., :C_] = core
        xt4[..., C_:C_ + 2] = np.float16(1.0)
        in_maps.append({
            "x16": np.ascontiguousarray(xi),
            "xt4": np.ascontiguousarray(xt4.reshape(128, (n // 128) * WPAD)),
            "w1t": w1t, "w2t": w2t, "b1": b1r, "b2": b2r,
        })
    return in_maps


def kernel(x, w1, b1, w2, b2):
    """Channel-attention forward for x:(8,256,128,128); returns same shape.

    Data-parallel over the batch: one batch element per NeuronCore.
    """
    from concourse.bass_utils import run_bass_kernel_spmd

    B, C_, H, W = x.shape
    nc = _get_nc()
    in_maps = _prepare_in_maps(x, w1, b1, w2, b2)
    res = run_bass_kernel_spmd(nc, in_maps, core_ids=list(range(B)))
    out = np.stack(
        [res.results[i]["y"].astype(np.float32) for i in range(B)], axis=0
    )
    return out.reshape(B, C_, H, W)
